# revision 13
# baseline (speedup 1.0000x reference)
"""PointPillarScatter3d on 8 Trainium2 NeuronCores (Bass/Tile).

kernel(pillar_features [N,64] f32, voxel_coords [N,4] i32 (b,z,y,x),
       batch_size () i64) -> (B, 128, 512, 512) f32
where out[b, 2c+z, y, x] = pillar_features[i, c] for each pillar i.

Sharding (data parallel, no cross-core comms): core k handles
(batch k>>1, z k&1) and produces shard [64, 512*512]; host assembles.

Device pipeline per 16384-position chunk:
  memset two SBUF accumulators [128, 64*64] ->
  dma_scatter_add (SBUF-dst, tokens_per_rank=128) places each pillar's
  64 features contiguously at [pos&127, group*64:...] of the own (first
  half of chunk) or peer buffer; -1 index suffix padding is skipped by
  the ucode, the valid count comes from a per-chunk count register ->
  PE transposes adjacent pair slices [128 pos, 2x64 ch] -> PSUM
  [128 (h*64+c), 128 pos], 4 pairs per [128,512] PSUM bank ->
  DVE/ACT copy PSUM -> wide [128, 4096] ->
  two [64, 4096] DMAs per wide (alternating sync/scalar HWDGE rings,
  outer dim 64 so the transfer sprays all 16 SDMA engines): partition c
  -> 16 KB contiguous run at channel plane c.
"""

import numpy as np

NX, NY, NZ = 512, 512, 2
NCH = 64
NPOS = NY * NX
CHUNK = 16384
KTOK_DEFAULT = 1536

_CACHE = {}


def _build_nc(npos, chunk, ktok, split_dma=True, runtime_cnt=True):
    import concourse.bacc as bacc
    import concourse.bass as bass
    import concourse.mybir as mybir
    import concourse.tile as tile
    from concourse.masks import make_identity

    F32 = mybir.dt.float32
    I16 = mybir.dt.int16
    I32 = mybir.dt.int32
    nchunks = npos // chunk
    G = chunk // 256
    G2 = G // 2
    quarter = chunk // 4
    kb = ktok // 128

    nc = bacc.Bacc("TRN2", target_bir_lowering=False)
    feats = nc.dram_tensor("feats", [nchunks, 128, kb * NCH], F32,
                           kind="ExternalInput")
    idxs = nc.dram_tensor("idxs", [128, nchunks * (ktok // 16)], I16,
                          kind="ExternalInput")
    cnts = nc.dram_tensor("cnts", [1, nchunks], I32, kind="ExternalInput")
    out = nc.dram_tensor("out", [NCH, npos], F32, kind="ExternalOutput")

    with tile.TileContext(nc) as tc:
        with (
            tc.tile_pool(name="const", bufs=1) as cpool,
            tc.tile_pool(name="idx", bufs=1) as ipool,
            tc.tile_pool(name="feat", bufs=4) as fpool,
            tc.tile_pool(name="acc", bufs=3) as bpool,
            tc.tile_pool(name="wide", bufs=4) as wpool,
            tc.tile_pool(name="ps", bufs=8, space="PSUM") as ppool,
        ):
            ident = cpool.tile([128, 128], F32)
            make_identity(nc, ident[:])
            idx_all = ipool.tile([128, nchunks * (ktok // 16)], I16)
            nc.sync.dma_start(out=idx_all[:], in_=idxs[:])
            cnt_all = ipool.tile([1, nchunks], I32)
            nc.sync.dma_start(out=cnt_all[:], in_=cnts[:])

            ring = [nc.sync, nc.scalar]
            nring = 0
            for k in range(nchunks):
                feat = fpool.tile([128, kb, NCH], F32, tag="feat")
                nc.sync.dma_start(out=feat[:], in_=feats[k])
                # one extra trash group: padding tokens (idx = G<<8) land
                # there and are never read by the transposes
                own = bpool.tile([128, (G + 1) * NCH], F32, tag="own")
                peer = bpool.tile([128, (G + 1) * NCH], F32, tag="peer")
                nc.scalar.memzero(own[:])
                nc.vector.memset(peer[:], 0.0)
                if runtime_cnt:
                    # no min/max bounds: s_runtime_assert surfaces as a fatal
                    # NRT notification on this runtime
                    cnt = nc.gpsimd.value_load(cnt_all[:1, k:k + 1])
                else:
                    cnt = ktok
                nc.gpsimd.dma_scatter_add(
                    own[:],
                    feat[:],
                    idx_all[:, k * (ktok // 16):(k + 1) * (ktok // 16)],
                    ktok,
                    cnt,
                    NCH,
                    sbuf_tokens_per_rank=128,
                    parity_reg=0,
                    out_ap_other=peer[:],
                )
                for half, buf in ((0, own), (1, peer)):
                    wide = wpool.tile([128, quarter], F32, tag="wide")
                    for pg in range(G2 // 4):
                        ps = ppool.tile([128, 512], F32)
                        for j in range(4):
                            p = pg * 4 + j
                            nc.tensor.transpose(
                                out=ps[:, j * 128:(j + 1) * 128],
                                in_=buf[:, 128 * p:128 * (p + 1)],
                                identity=ident[:],
                            )
                        dst = wide[:, pg * 512:(pg + 1) * 512]
                        if pg % 2 == 0:
                            nc.vector.tensor_copy(out=dst, in_=ps[:])
                        else:
                            nc.scalar.copy(dst, ps[:])
                    base = k * chunk + half * 2 * quarter
                    full = out[:]
                    if split_dma:
                        for h2 in (0, 1):
                            dram_ap = bass.AP(
                                full.tensor, base + h2 * quarter,
                                [[npos, NCH], [1, quarter]],
                            )
                            eng = ring[nring % 2]
                            nring += 1
                            eng.dma_start(
                                out=dram_ap, in_=wide[64 * h2:64 * (h2 + 1), :])
                    else:
                        dram_ap = bass.AP(
                            full.tensor, base,
                            [[quarter, 2], [npos, NCH], [1, quarter]],
                        )
                        nc.sync.dma_start(out=dram_ap, in_=wide[:])
    nc.compile()
    return nc


def _pack_core(coords_s, feats_np, npos, chunk, ktok, neg_pad=True):
    """coords_s: positions (y*NX+x) of this core's pillars; feats [n, 64]."""
    nchunks = npos // chunk
    G = chunk // 256
    kb = ktok // 128
    order = np.argsort(coords_s, kind="stable")
    s = coords_s[order]
    f = feats_np[order]
    bins = (s // chunk).astype(np.int64)
    starts = np.searchsorted(bins, np.arange(nchunks))
    ends = np.searchsorted(bins, np.arange(nchunks) + 1)
    counts = (ends - starts).astype(np.int32)
    if counts.max(initial=0) > ktok:
        raise OverflowError(f"chunk overflow: {counts.max()} > {ktok}")

    feat_pack = np.zeros((nchunks, ktok, NCH), np.float32)
    if neg_pad:
        idx_pack = np.full((nchunks, ktok), -1, np.int16)  # -1 suffix: skipped
    else:
        # padding tokens go to the dedicated trash group (never read);
        # they must not hit a real position: concurrent CCE read-modify-
        # write adds from different SDMA engines lose updates
        idx_pack = np.full((nchunks, ktok), G << 8, np.int16)
    local = s % chunk
    t = local >> 7
    p = local & 127
    half = t // G
    u = t % G
    # group permutation: pair tiles (u, u+G/2) sit in adjacent groups
    # (2u', 2u'+1) so each PE-transpose pair is one contiguous slice
    g = np.where(u < G // 2, 2 * u, 2 * (u - G // 2) + 1)
    idxv = ((g << 8) | (half << 7) | p).astype(np.int16)
    for k in range(nchunks):
        n = counts[k]
        if n:
            feat_pack[k, :n] = f[starts[k]:ends[k]]
            idx_pack[k, :n] = idxv[starts[k]:ends[k]]
    feats_dev = (
        feat_pack.reshape(nchunks, kb, 128, NCH)
        .swapaxes(1, 2)
        .reshape(nchunks, 128, kb * NCH)
        .copy()
    )
    idxs_dev = (
        idx_pack.reshape(nchunks, ktok // 16, 16)
        .swapaxes(1, 2)
        .reshape(nchunks, 16, ktok // 16)
        .transpose(1, 0, 2)
        .reshape(16, nchunks * (ktok // 16))
    )
    # 16-partition pattern replicated 8x (one copy per GpSimd Q7 core)
    idxs_dev = np.tile(idxs_dev, (8, 1)).copy()
    return feats_dev, idxs_dev, counts.reshape(1, nchunks)


def _numpy_fallback(pillar_features, voxel_coords, batch_size):
    c = np.asarray(voxel_coords).astype(np.int64)
    f = np.asarray(pillar_features, np.float32)
    out = np.zeros((batch_size, NZ * NY * NX, NCH), np.float32)
    sp = c[:, 1] * (NY * NX) + c[:, 2] * NX + c[:, 3]
    out[c[:, 0], sp] = f
    return out.transpose(0, 2, 1).reshape(batch_size, NCH * NZ, NY, NX)


def make_in_maps(pillar_features, voxel_coords, npos, chunk, ktok):
    pf = np.asarray(pillar_features, np.float32)
    vc = np.asarray(voxel_coords)
    s_all = vc[:, 2].astype(np.int64) * NX + vc[:, 3].astype(np.int64)
    core_of = vc[:, 0].astype(np.int64) * 2 + vc[:, 1].astype(np.int64)
    in_maps = []
    for k in range(8):
        m = core_of == k
        fd, xd, cn = _pack_core(s_all[m], pf[m], npos, chunk, ktok)
        in_maps.append({"feats": fd, "idxs": xd, "cnts": cn})
    return in_maps


def assemble(results, batch_size=4):
    full = np.empty((batch_size, NCH, NZ, NY, NX), np.float32)
    for k in range(2 * batch_size):
        full[k >> 1, :, k & 1] = results[k]["out"].reshape(NCH, NY, NX)
    return full.reshape(batch_size, NCH * NZ, NY, NX)


def kernel(pillar_features, voxel_coords, batch_size):
    b = int(np.asarray(batch_size))
    pf = np.asarray(pillar_features, np.float32)
    vc = np.asarray(voxel_coords)
    if b != 4 or pf.shape[1] != NCH:
        return _numpy_fallback(pf, vc, b)

    ktok = KTOK_DEFAULT
    while True:
        try:
            in_maps = make_in_maps(pf, vc, NPOS, CHUNK, ktok)
            break
        except OverflowError:
            ktok *= 2
            if ktok > 32768:
                return _numpy_fallback(pf, vc, b)

    key = (NPOS, CHUNK, ktok)
    if key not in _CACHE:
        _CACHE[key] = _build_nc(*key)
    nc = _CACHE[key]

    from concourse.bass_utils import run_bass_kernel_spmd

    res = run_bass_kernel_spmd(nc, in_maps, core_ids=list(range(8)))
    return assemble(res.results, b)


# revision 15
# speedup vs baseline: 1.0844x; 1.0844x over previous
"""PointPillarScatter3d on 8 Trainium2 NeuronCores (Bass/Tile).

kernel(pillar_features [N,64] f32, voxel_coords [N,4] i32 (b,z,y,x),
       batch_size () i64) -> (B, 128, 512, 512) f32
where out[b, 2c+z, y, x] = pillar_features[i, c] for each pillar i.

Sharding (data parallel, no cross-core comms): core k handles
(batch k>>1, z k&1) and produces shard [64, 512*512]; host assembles.

Device pipeline per 16384-position chunk:
  memset two SBUF accumulators [128, 64*64] ->
  dma_scatter_add (SBUF-dst, tokens_per_rank=128) places each pillar's
  64 features contiguously at [pos&127, group*64:...] of the own (first
  half of chunk) or peer buffer; -1 index suffix padding is skipped by
  the ucode, the valid count comes from a per-chunk count register ->
  PE transposes adjacent pair slices [128 pos, 2x64 ch] -> PSUM
  [128 (h*64+c), 128 pos], 4 pairs per [128,512] PSUM bank ->
  DVE/ACT copy PSUM -> wide [128, 4096] ->
  two [64, 4096] DMAs per wide (alternating sync/scalar HWDGE rings,
  outer dim 64 so the transfer sprays all 16 SDMA engines): partition c
  -> 16 KB contiguous run at channel plane c.
"""

import numpy as np

NX, NY, NZ = 512, 512, 2
NCH = 64
NPOS = NY * NX
CHUNK = 16384
KTOK_DEFAULT = 1536

_CACHE = {}


def _build_nc(npos, chunk, ktok, split_dma=True, runtime_cnt=True):
    import concourse.bacc as bacc
    import concourse.bass as bass
    import concourse.mybir as mybir
    import concourse.tile as tile
    from concourse.masks import make_identity

    F32 = mybir.dt.float32
    I16 = mybir.dt.int16
    I32 = mybir.dt.int32
    nchunks = npos // chunk
    G = chunk // 256
    G2 = G // 2
    quarter = chunk // 4
    kb = ktok // 128

    nc = bacc.Bacc("TRN2", target_bir_lowering=False)
    feats = nc.dram_tensor("feats", [nchunks, 128, kb * NCH], F32,
                           kind="ExternalInput")
    idxs = nc.dram_tensor("idxs", [128, nchunks * (ktok // 16)], I16,
                          kind="ExternalInput")
    cnts = nc.dram_tensor("cnts", [1, nchunks], I32, kind="ExternalInput")
    out = nc.dram_tensor("out", [NCH, npos], F32, kind="ExternalOutput")

    with tile.TileContext(nc) as tc:
        with (
            tc.tile_pool(name="const", bufs=1) as cpool,
            tc.tile_pool(name="idx", bufs=1) as ipool,
            tc.tile_pool(name="feat", bufs=4) as fpool,
            tc.tile_pool(name="acc", bufs=3) as bpool,
            tc.tile_pool(name="wide", bufs=4) as wpool,
            tc.tile_pool(name="ps", bufs=8, space="PSUM") as ppool,
        ):
            ident = cpool.tile([128, 128], F32)
            make_identity(nc, ident[:])
            idx_all = ipool.tile([128, nchunks * (ktok // 16)], I16)
            nc.sync.dma_start(out=idx_all[:], in_=idxs[:])
            cnt_all = ipool.tile([1, nchunks], I32)
            nc.sync.dma_start(out=cnt_all[:], in_=cnts[:])

            ring = [nc.sync, nc.scalar, nc.gpsimd]
            nring = 0
            for k in range(nchunks):
                feat = fpool.tile([128, kb, NCH], F32, tag="feat")
                nc.sync.dma_start(out=feat[:], in_=feats[k])
                # one extra trash group: padding tokens (idx = G<<8) land
                # there and are never read by the transposes
                own = bpool.tile([128, (G + 1) * NCH], F32, tag="own")
                peer = bpool.tile([128, (G + 1) * NCH], F32, tag="peer")
                nc.scalar.memzero(own[:])
                nc.vector.memset(peer[:], 0.0)
                if runtime_cnt:
                    # no min/max bounds: s_runtime_assert surfaces as a fatal
                    # NRT notification on this runtime
                    cnt = nc.gpsimd.value_load(cnt_all[:1, k:k + 1])
                else:
                    cnt = ktok
                nc.gpsimd.dma_scatter_add(
                    own[:],
                    feat[:],
                    idx_all[:, k * (ktok // 16):(k + 1) * (ktok // 16)],
                    ktok,
                    cnt,
                    NCH,
                    sbuf_tokens_per_rank=128,
                    parity_reg=0,
                    out_ap_other=peer[:],
                )
                for half, buf in ((0, own), (1, peer)):
                    wide = wpool.tile([128, quarter], F32, tag="wide")
                    for pg in range(G2 // 4):
                        ps = ppool.tile([128, 512], F32)
                        for j in range(4):
                            p = pg * 4 + j
                            nc.tensor.transpose(
                                out=ps[:, j * 128:(j + 1) * 128],
                                in_=buf[:, 128 * p:128 * (p + 1)],
                                identity=ident[:],
                            )
                        dst = wide[:, pg * 512:(pg + 1) * 512]
                        if pg % 2 == 0:
                            nc.vector.tensor_copy(out=dst, in_=ps[:])
                        else:
                            nc.scalar.copy(dst, ps[:])
                    base = k * chunk + half * 2 * quarter
                    full = out[:]
                    if split_dma:
                        for h2 in (0, 1):
                            dram_ap = bass.AP(
                                full.tensor, base + h2 * quarter,
                                [[npos, NCH], [1, quarter]],
                            )
                            eng = ring[nring % len(ring)]
                            nring += 1
                            eng.dma_start(
                                out=dram_ap, in_=wide[64 * h2:64 * (h2 + 1), :])
                    else:
                        dram_ap = bass.AP(
                            full.tensor, base,
                            [[quarter, 2], [npos, NCH], [1, quarter]],
                        )
                        nc.sync.dma_start(out=dram_ap, in_=wide[:])
    nc.compile()
    return nc


def _pack_core(coords_s, feats_np, npos, chunk, ktok, neg_pad=True):
    """coords_s: positions (y*NX+x) of this core's pillars; feats [n, 64]."""
    nchunks = npos // chunk
    G = chunk // 256
    kb = ktok // 128
    order = np.argsort(coords_s, kind="stable")
    s = coords_s[order]
    f = feats_np[order]
    bins = (s // chunk).astype(np.int64)
    starts = np.searchsorted(bins, np.arange(nchunks))
    ends = np.searchsorted(bins, np.arange(nchunks) + 1)
    counts = (ends - starts).astype(np.int32)
    if counts.max(initial=0) > ktok:
        raise OverflowError(f"chunk overflow: {counts.max()} > {ktok}")

    feat_pack = np.zeros((nchunks, ktok, NCH), np.float32)
    if neg_pad:
        idx_pack = np.full((nchunks, ktok), -1, np.int16)  # -1 suffix: skipped
    else:
        # padding tokens go to the dedicated trash group (never read);
        # they must not hit a real position: concurrent CCE read-modify-
        # write adds from different SDMA engines lose updates
        idx_pack = np.full((nchunks, ktok), G << 8, np.int16)
    local = s % chunk
    t = local >> 7
    p = local & 127
    half = t // G
    u = t % G
    # group permutation: pair tiles (u, u+G/2) sit in adjacent groups
    # (2u', 2u'+1) so each PE-transpose pair is one contiguous slice
    g = np.where(u < G // 2, 2 * u, 2 * (u - G // 2) + 1)
    idxv = ((g << 8) | (half << 7) | p).astype(np.int16)
    for k in range(nchunks):
        n = counts[k]
        if n:
            feat_pack[k, :n] = f[starts[k]:ends[k]]
            idx_pack[k, :n] = idxv[starts[k]:ends[k]]
    feats_dev = (
        feat_pack.reshape(nchunks, kb, 128, NCH)
        .swapaxes(1, 2)
        .reshape(nchunks, 128, kb * NCH)
        .copy()
    )
    idxs_dev = (
        idx_pack.reshape(nchunks, ktok // 16, 16)
        .swapaxes(1, 2)
        .reshape(nchunks, 16, ktok // 16)
        .transpose(1, 0, 2)
        .reshape(16, nchunks * (ktok // 16))
    )
    # 16-partition pattern replicated 8x (one copy per GpSimd Q7 core)
    idxs_dev = np.tile(idxs_dev, (8, 1)).copy()
    return feats_dev, idxs_dev, counts.reshape(1, nchunks)


def _numpy_fallback(pillar_features, voxel_coords, batch_size):
    c = np.asarray(voxel_coords).astype(np.int64)
    f = np.asarray(pillar_features, np.float32)
    out = np.zeros((batch_size, NZ * NY * NX, NCH), np.float32)
    sp = c[:, 1] * (NY * NX) + c[:, 2] * NX + c[:, 3]
    out[c[:, 0], sp] = f
    return out.transpose(0, 2, 1).reshape(batch_size, NCH * NZ, NY, NX)


def make_in_maps(pillar_features, voxel_coords, npos, chunk, ktok):
    pf = np.asarray(pillar_features, np.float32)
    vc = np.asarray(voxel_coords)
    s_all = vc[:, 2].astype(np.int64) * NX + vc[:, 3].astype(np.int64)
    core_of = vc[:, 0].astype(np.int64) * 2 + vc[:, 1].astype(np.int64)
    in_maps = []
    for k in range(8):
        m = core_of == k
        fd, xd, cn = _pack_core(s_all[m], pf[m], npos, chunk, ktok)
        in_maps.append({"feats": fd, "idxs": xd, "cnts": cn})
    return in_maps


def assemble(results, batch_size=4):
    full = np.empty((batch_size, NCH, NZ, NY, NX), np.float32)
    for k in range(2 * batch_size):
        full[k >> 1, :, k & 1] = results[k]["out"].reshape(NCH, NY, NX)
    return full.reshape(batch_size, NCH * NZ, NY, NX)


def kernel(pillar_features, voxel_coords, batch_size):
    b = int(np.asarray(batch_size))
    pf = np.asarray(pillar_features, np.float32)
    vc = np.asarray(voxel_coords)
    if b != 4 or pf.shape[1] != NCH:
        return _numpy_fallback(pf, vc, b)

    ktok = KTOK_DEFAULT
    while True:
        try:
            in_maps = make_in_maps(pf, vc, NPOS, CHUNK, ktok)
            break
        except OverflowError:
            ktok *= 2
            if ktok > 32768:
                return _numpy_fallback(pf, vc, b)

    key = (NPOS, CHUNK, ktok)
    if key not in _CACHE:
        _CACHE[key] = _build_nc(*key)
    nc = _CACHE[key]

    from concourse.bass_utils import run_bass_kernel_spmd

    res = run_bass_kernel_spmd(nc, in_maps, core_ids=list(range(8)))
    return assemble(res.results, b)
